# revision 1
# baseline (speedup 1.0000x reference)
"""Trainium2 Bass kernel for nn_MultiHeadAttention_52862457480066.

Reference computation (B=2, N=1024, D=512, H=16, DH=32):
    qkv = x @ att_w.T ; q,k,v per head
    score = q.k/sqrt(DH) - gamma_p*pdist + angle@w_bias.T + gamma_adj*adj
    score = where(mask, -1e9, score) ; prob = softmax_j(score)
    att = prob @ v ; ff = att @ ff_w.T + ff_b ; y = x + ff ; out = LayerNorm(y)*ln_w+ln_b

Sharding over 8 cores: (batch b in 2) x (query-half ih in 2) x (head-half hg in 2).
Each core: 8 heads, 512 query rows (i), all 1024 key rows (j).

Key structure:
- Scores are computed TRANSPOSED ([j_part, i_free]) so softmax'd probs feed the
  attention*V matmul directly as the moving operand; the host pre-transposes
  inputs while slicing shards (part of the sharding layout choice).
- All score-bias terms are accumulated into the score PSUM by the PE itself:
  a plain identity matmul adds the head-shared P0 = gamma_adj*adj -
  gamma_p*pdist (gammas are head-uniform for this module's inputs; a general
  fallback path handles non-uniform gammas on the DVE), and per-head scaled
  identities (identity * w_bias[h,c], built on-device) add the two angle
  features. exp() then reads PSUM directly on the scalar engine, so the hot
  4.2M-element softmax path needs zero vector-engine passes.
- Softmax denominators come from N=1 matmuls (ones moving operand) giving
  rowsums in [i_part, head_free] layout where the iterative reciprocal is
  cheap; normalization is deferred to after the AV matmul (divides 16*512
  values per core instead of 4.2M).
- PSUM accumulators written by interleaved matmul chains (AV col-groups,
  rowsum columns) are zero-initialized by one full-coverage start=True matmul;
  everything after runs start=False using per-element has_written semantics.
- Matmul operands are float32r (full PE rate; plain fp32 is 4x slower).
- The FF projection is computed per-head-group and pair-ReduceScattered
  (cores 2k<->2k+1) before the residual+LayerNorm epilogue.
"""

import math

import numpy as np

import concourse.bass as bass
import concourse.tile as tile
from concourse import bacc, mybir
from concourse.bass_utils import run_bass_kernel_spmd
from concourse.masks import make_identity

B, N, D, H, DH = 2, 1024, 512, 16, 32
NH = H // 2          # heads per core
NI = N // 2          # query rows per core
NJT = N // 128       # key tiles (partition dim j)
NEG_INF = -1e9
LN_EPS = 1e-5
QSCALE = 1.0 / math.sqrt(DH)
F32 = mybir.dt.float32
F32R = mybir.dt.float32r
BF16 = mybir.dt.bfloat16
N_CORES = 8
RS_GROUPS = [[0, 1], [2, 3], [4, 5], [6, 7]]

DEBUG_TAPS = False   # add intermediate-dump outputs (debugging only)


def _r(ap):
    return ap.bitcast(F32R)


def build_program(uniform: bool, gp: float, ga: float, trivial_ln: bool = False,
                  no_collective: bool = False):
    """Build the SPMD bass program (identical on all 8 cores)."""
    nc = bacc.Bacc("TRN2", target_bir_lowering=False, debug=False, num_devices=N_CORES)

    # ---- DRAM I/O (per-core views, host-sharded) ----
    d_xT = nc.dram_tensor("xT", [D, N], F32, kind="ExternalInput").ap()        # x[b].T
    d_xiT = nc.dram_tensor("xiT", [D, NI], F32, kind="ExternalInput").ap()     # x[b,irows].T
    d_xrows = nc.dram_tensor("xrows", [256, D], F32, kind="ExternalInput").ap()
    d_wqkT = nc.dram_tensor("wqkT", [D, 512], F32, kind="ExternalInput").ap()  # [d, 256 q | 256 k]
    d_wvT = nc.dram_tensor("wvT", [D, 256], F32, kind="ExternalInput").ap()
    d_ffwT = nc.dram_tensor("ffwT", [256, D], F32, kind="ExternalInput").ap()
    d_pdT = nc.dram_tensor("pdT", [N, NI], F32, kind="ExternalInput").ap()
    d_adT = nc.dram_tensor("adT", [N, NI], F32, kind="ExternalInput").ap()
    d_a0T = nc.dram_tensor("a0T", [N, NI], F32, kind="ExternalInput").ap()
    d_a1T = nc.dram_tensor("a1T", [N, NI], F32, kind="ExternalInput").ap()
    h_hcoef = nc.dram_tensor("hcoef", [NH, 4], F32, kind="ExternalInput")      # w0,w1,-gp,ga
    d_maskb = nc.dram_tensor("maskb", [N], F32, kind="ExternalInput").ap()
    d_lnw = nc.dram_tensor("lnw", [D], F32, kind="ExternalInput").ap()
    d_lnb = nc.dram_tensor("lnb", [D], F32, kind="ExternalInput").ap()
    d_ffb = nc.dram_tensor("ffb", [D], F32, kind="ExternalInput").ap()
    d_ffpart = nc.dram_tensor("ffpart", [NI, D], F32).ap()
    d_ffrs = nc.dram_tensor("ffrs", [256, D], F32).ap()
    d_out = nc.dram_tensor("out", [256, D], F32, kind="ExternalOutput").ap()
    taps = {}
    if DEBUG_TAPS:
        for nm, shp in [("tap_ffpart", [NI, D]), ("tap_qT", [128, 2, NI]),
                        ("tap_kT", [128, 2, N]), ("tap_v", [128, NJT, 256]),
                        ("tap_P0", [128, NJT, NI]), ("tap_attn", [128, 2, NI]),
                        ("tap_rs", [2, 128, 16]), ("tap_p", [128, NI]),
                        ("tap_sc", [128, NI])]:
            taps[nm] = nc.dram_tensor(nm, shp, F32, kind="ExternalOutput").ap()

    with tile.TileContext(nc) as tc:
        _emit(nc, tc, locals(), uniform, gp, ga, trivial_ln, taps, no_collective)
    nc.compile()
    return nc


def _emit(nc, tc, t, uniform, gp, ga, trivial_ln=False, taps=None, no_collective=False):
    taps = taps or {}
    AL = mybir.AluOpType
    AF = mybir.ActivationFunctionType
    from contextlib import ExitStack

    ctx = ExitStack()
    with ctx:
        consts = ctx.enter_context(tc.tile_pool(name="consts", bufs=1))
        big = ctx.enter_context(tc.tile_pool(name="big", bufs=1))
        stream = ctx.enter_context(tc.tile_pool(name="stream", bufs=6))
        tiny = ctx.enter_context(tc.tile_pool(name="tiny", bufs=8))
        ppool = ctx.enter_context(tc.tile_pool(name="ppool", bufs=6))
        ps_mm = ctx.enter_context(tc.tile_pool(name="ps_mm", bufs=4, space="PSUM"))
        ps_sc = ps_mm
        ps_av = ctx.enter_context(tc.tile_pool(name="ps_av", bufs=4, space="PSUM"))
        ps_rs = ps_av
        proj_ctx = ExitStack()
        proj = proj_ctx.enter_context(tc.tile_pool(name="proj", bufs=1))

        # ---------------- constants / small tiles ----------------
        identity_f = consts.tile([128, 128], F32)  # for PE transposes (f32 path)
        make_identity(nc, identity_f[:])
        identity = consts.tile([128, 128], F32R)   # for PSUM bias-adds (f32r path)
        nc.vector.tensor_copy(identity[:], identity_f[:])
        ind4 = consts.tile([4, 128], F32)  # ind4[k, m] = (m//32 == k)
        nc.gpsimd.memset(ind4[:], 1.0)
        nc.gpsimd.affine_select(
            out=ind4[:], in_=ind4[:], compare_op=AL.is_ge, fill=0.0,
            base=0, pattern=[[1, 128]], channel_multiplier=-32,
        )
        nc.gpsimd.affine_select(
            out=ind4[:], in_=ind4[:], compare_op=AL.is_ge, fill=0.0,
            base=31, pattern=[[-1, 128]], channel_multiplier=32,
        )
        ones_col = consts.tile([128, 1], BF16)
        nc.gpsimd.memset(ones_col[:], 1.0)
        ones_row_f = consts.tile([1, 128], F32)
        nc.gpsimd.memset(ones_row_f[:], 1.0)
        zeros_row_f = consts.tile([1, 512], F32)
        nc.gpsimd.memset(zeros_row_f[:], 0.0)
        ones_row = consts.tile([1, 128], F32R)
        nc.vector.tensor_copy(ones_row[:], ones_row_f[:])
        zeros_row = consts.tile([1, 512], F32R)
        nc.vector.tensor_copy(zeros_row[:], zeros_row_f[:])
        ones_row_b = consts.tile([1, 128], BF16)
        nc.vector.tensor_copy(ones_row_b[:], ones_row_f[:])
        zeros_row_b = consts.tile([1, 512], BF16)
        nc.vector.tensor_copy(zeros_row_b[:], zeros_row_f[:])

        maskb = consts.tile([128, NJT], F32)
        nc.gpsimd.dma_start(out=maskb[:], in_=t["d_maskb"].rearrange("(t p) -> p t", p=128))
        hc = t["h_hcoef"]
        hbc = []  # [w0, w1, -gp, ga] broadcast [128, NH]
        for c in range(4):
            bc = consts.tile([128, NH], F32, tag=f"hbc{c}")
            nc.gpsimd.dma_start(
                out=bc[:], in_=bass.AP(tensor=hc, offset=c, ap=[[0, 128], [4, NH]])
            )
            hbc.append(bc)
        w0bc, w1bc, ngpbc, gabc = hbc

        # per-head scaled identities for the angle-feature PSUM adds
        idw = []  # idw[c][hl] = identity * w_bias[head, c]
        for c, wbc in ((0, w0bc), (1, w1bc)):
            row = []
            for hl in range(NH):
                it_ = consts.tile([128, 128], F32R, tag=f"idw{c}_{hl}")
                nc.vector.tensor_scalar(
                    it_[:], identity[:], wbc[:, hl : hl + 1], None, AL.mult
                )
                row.append(it_)
            idw.append(row)

        lnw_row = consts.tile([1, D], F32)
        nc.gpsimd.dma_start(out=lnw_row[:], in_=t["d_lnw"].rearrange("(o d) -> o d", o=1))
        lnb_row = consts.tile([1, D], F32)
        nc.gpsimd.dma_start(out=lnb_row[:], in_=t["d_lnb"].rearrange("(o d) -> o d", o=1))
        ffb_row = consts.tile([1, D], F32R)
        nc.gpsimd.dma_start(out=ffb_row[:], in_=_r(t["d_ffb"]).rearrange("(o d) -> o d", o=1))

        # broadcast ln_w/ln_b across partitions via K=1 matmul
        lnw_bc = lnb_bc = None
        if not trivial_ln:
            lnw_bc = consts.tile([128, D], F32)
            lnb_bc = consts.tile([128, D], F32)
            for row, bc in ((lnw_row, lnw_bc), (lnb_row, lnb_bc)):
                ps = ps_mm.tile([128, D], F32, tag="mmps")
                nc.tensor.matmul(ps[:], ones_row_f[0:1, :], row[0:1, :], start=True, stop=True)
                nc.vector.tensor_copy(bc[:], ps[:])

        # ---------------- load big inputs ----------------
        wqkT = proj.tile([128, 4, 512], F32R)
        nc.sync.dma_start(out=wqkT[:], in_=_r(t["d_wqkT"]).rearrange("(c p) f -> p c f", p=128))
        xiT = proj.tile([128, 4, NI], F32R)
        nc.sync.dma_start(out=xiT[:], in_=_r(t["d_xiT"]).rearrange("(c p) n -> p c n", p=128))
        xT = proj.tile([128, 4, N], F32R)
        nc.sync.dma_start(out=xT[:], in_=_r(t["d_xT"]).rearrange("(c p) n -> p c n", p=128))
        wvT = proj.tile([128, 4, 256], F32R)
        nc.sync.dma_start(out=wvT[:], in_=_r(t["d_wvT"]).rearrange("(c p) f -> p c f", p=128))

        # ---------------- q/k projection (transposed: [feat, n]) ----------------
        # wqkT f-cols: [0:256) = q rows of att_w (local heads 0..8), [256:512) = k
        qT = big.tile([128, 2, NI], F32R)   # [dh-part(4h), qtile, i]
        kT = big.tile([128, 2, N], F32R)    # [dh-part(4h), ktile, n]
        for ft in range(4):
            is_q = ft < 2
            nch = 1 if is_q else 2
            for nc_i in range(nch):
                ps = ps_mm.tile([128, 512], F32, tag="mmps")
                for dc in range(4):
                    rhs_src = xiT if is_q else xT
                    rhs = rhs_src[:, dc, nc_i * 512 : nc_i * 512 + 512]
                    lhsT = wqkT[:, dc, ft * 128 : (ft + 1) * 128]
                    nc.tensor.matmul(ps[:], lhsT, rhs, start=(dc == 0), stop=(dc == 3))
                if is_q:
                    nc.vector.tensor_scalar(
                        qT[:, ft, :], ps[:, 0:NI], QSCALE, None, AL.mult
                    )
                else:
                    nc.vector.tensor_copy(
                        kT[:, ft - 2, nc_i * 512 : nc_i * 512 + 512], ps[:]
                    )

        # ---------------- v projection (natural: [n, feat]) ----------------
        v = big.tile([128, NJT, 256], BF16)  # [j-part, jt, 8h*32]
        for nt in range(NJT):
            ps = ps_mm.tile([128, 256], F32, tag="mmps")
            for dc in range(4):
                lhsT = xT[:, dc, nt * 128 : (nt + 1) * 128]
                nc.tensor.matmul(ps[:], lhsT, wvT[:, dc, :], start=(dc == 0), stop=(dc == 3))
            nc.scalar.copy(v[:, nt, :], ps[:])
        proj_ctx.close()

        # ---------------- features + shared bias P0 = ga*adj - gp*pdist ----------
        # per-jt loads so wave-0 can start as soon as the first slices land
        a0 = big.tile([128, NJT, NI], F32R)
        a1 = big.tile([128, NJT, NI], F32R)
        P0 = None
        pd_tiles = [None] * NJT
        ad_tiles = [None] * NJT
        if uniform:
            P0 = big.tile([128, NJT, NI], F32R)
            for jt in range(NJT):
                pd = stream.tile([128, NI], F32, tag="t512")
                nc.sync.dma_start(out=pd[:], in_=t["d_pdT"][jt * 128 : (jt + 1) * 128, :])
                ad = stream.tile([128, NI], F32, tag="t512")
                nc.sync.dma_start(out=ad[:], in_=t["d_adT"][jt * 128 : (jt + 1) * 128, :])
                nc.sync.dma_start(
                    out=a0[:, jt, :], in_=_r(t["d_a0T"][jt * 128 : (jt + 1) * 128, :])
                )
                nc.sync.dma_start(
                    out=a1[:, jt, :], in_=_r(t["d_a1T"][jt * 128 : (jt + 1) * 128, :])
                )
                tmp = stream.tile([128, NI], F32, tag="t512")
                nc.vector.tensor_scalar(tmp[:], ad[:], float(ga), None, AL.mult)
                nc.vector.scalar_tensor_tensor(
                    P0[:, jt, :], pd[:], float(-gp), tmp[:], AL.mult, AL.add
                )
        else:
            # general per-head gammas: keep pdist/adj resident, fold per head (DVE)
            for jt in range(NJT):
                pd = big.tile([128, NI], F32, tag=f"pdr{jt}")
                nc.sync.dma_start(out=pd[:], in_=t["d_pdT"][jt * 128 : (jt + 1) * 128, :])
                ad = big.tile([128, NI], F32, tag=f"adr{jt}")
                nc.sync.dma_start(out=ad[:], in_=t["d_adT"][jt * 128 : (jt + 1) * 128, :])
                nc.sync.dma_start(
                    out=a0[:, jt, :], in_=_r(t["d_a0T"][jt * 128 : (jt + 1) * 128, :])
                )
                nc.sync.dma_start(
                    out=a1[:, jt, :], in_=_r(t["d_a1T"][jt * 128 : (jt + 1) * 128, :])
                )
                pd_tiles[jt], ad_tiles[jt] = pd, ad
        ffwT = big.tile([128, 2, D], F32R)
        nc.sync.dma_start(out=ffwT[:], in_=_r(t["d_ffwT"]).rearrange("(c p) d -> p c d", p=128))

        # ---------------- attention: 2 waves of 4 heads ----------------
        attn = big.tile([128, 2, NI], F32R)  # normalized att.T  [4h*32dh, wave, i]
        for w in range(2):
            av_ps = ps_av.tile([128, NI], F32, tag="avps")
            rs_ps = ps_av.tile([128, 16], F32, tag="avps")
            # zero-init accumulator banks (see module docstring)
            nc.tensor.matmul(
                av_ps[:], ones_row_b[0:1, :], zeros_row_b[0:1, 0:NI],
                start=True, stop=False, skip_group_check=True,
            )
            nc.tensor.matmul(
                rs_ps[:], ones_row_b[0:1, :], zeros_row_b[0:1, 0:16],
                start=True, stop=False, skip_group_check=True,
            )
            for jt in range(NJT):
                p_tiles = []
                for hh in range(4):
                    hl = w * 4 + hh
                    sc = ps_sc.tile([128, NI], F32, tag="mmps")
                    lhsT = kT[hh * 32 : (hh + 1) * 32, w, jt * 128 : (jt + 1) * 128]
                    rhs = qT[hh * 32 : (hh + 1) * 32, w, :]
                    nc.tensor.matmul(
                        sc[:], lhsT, rhs, start=True, stop=False,
                        tile_position=(hh * 32, 0),
                    )
                    nc.tensor.matmul(
                        sc[:], idw[0][hl][:], a0[:, jt, :], start=False, stop=False,
                    )
                    nc.tensor.matmul(
                        sc[:], idw[1][hl][:], a1[:, jt, :], start=False, stop=True,
                    )
                    if uniform:
                        # P0 add on the DVE (PE is the busier engine): fused
                        # with the PSUM evacuation the exp would otherwise do.
                        xs = stream.tile([128, NI], F32, tag="xs")
                        nc.vector.scalar_tensor_tensor(
                            xs[:], P0[:, jt, :], 1.0, sc[:], AL.mult, AL.add
                        )
                        xin = xs
                    else:
                        g1 = stream.tile([128, NI], F32, tag="t512")
                        nc.vector.scalar_tensor_tensor(
                            g1[:], pd_tiles[jt][:], ngpbc[:, hl : hl + 1], sc[:],
                            AL.mult, AL.add,
                        )
                        g2 = stream.tile([128, NI], F32, tag="t512")
                        nc.vector.scalar_tensor_tensor(
                            g2[:], ad_tiles[jt][:], gabc[:, hl : hl + 1], g1[:],
                            AL.mult, AL.add,
                        )
                        xin = g2
                    pT = ppool.tile([128, NI], BF16, tag="pT")
                    nc.scalar.activation(
                        pT[:], xin[:], AF.Exp, bias=maskb[:, jt : jt + 1], scale=1.0
                    )
                    p_tiles.append(pT)
                    if taps and w == 0 and jt == 0 and hh == 0:
                        pv = stream.tile([128, NI], F32, tag="t512")
                        nc.vector.tensor_copy(pv[:], pT[:])
                        nc.sync.dma_start(out=taps["tap_p"], in_=pv[:])
                        scv = stream.tile([128, NI], F32, tag="t512")
                        nc.scalar.copy(scv[:], sc[:])
                        nc.sync.dma_start(out=taps["tap_sc"], in_=scv[:])
                for hh in range(4):
                    pT = p_tiles[hh]
                    vcol = (w * 4 + hh) * 32
                    nc.tensor.matmul(
                        av_ps[hh * 32 : (hh + 1) * 32, :],
                        v[:, jt, vcol : vcol + 32],
                        pT[:],
                        start=False, stop=(jt == NJT - 1 and hh == 3),
                        tile_position=(0, hh * 32),
                        skip_group_check=True,
                    )
                    for ic in range(4):
                        col = ic * 4 + hh
                        nc.tensor.matmul(
                            rs_ps[:, col : col + 1],
                            pT[:, ic * 128 : (ic + 1) * 128],
                            ones_col[:],
                            start=False,
                            stop=(jt == NJT - 1 and hh == 3 and ic == 3),
                            skip_group_check=True,
                        )
            # normalize: attn = av / rowsum
            rs_sb = stream.tile([128, 16], F32, tag="t512")
            nc.vector.tensor_copy(rs_sb[:], rs_ps[:])
            if taps:
                nc.sync.dma_start(out=taps["tap_rs"][w], in_=rs_sb[:])
            recip = stream.tile([128, 16], F32, tag="t512")
            nc.vector.reciprocal(recip[:], rs_sb[:])
            recipT = stream.tile([4, NI], F32, tag="t512")
            for ic in range(4):
                trp = ps_mm.tile([4, 128], F32, tag="mmps")
                nc.tensor.transpose(trp[:], recip[:, ic * 4 : (ic + 1) * 4], identity_f[:])
                nc.vector.tensor_copy(recipT[:, ic * 128 : (ic + 1) * 128], trp[:])
            rbc_ps = ps_mm.tile([128, NI], F32, tag="mmps")
            nc.tensor.matmul(rbc_ps[:], ind4[:], recipT[:], start=True, stop=True)
            rbc = stream.tile([128, NI], F32, tag="t512")
            nc.vector.tensor_copy(rbc[:], rbc_ps[:])
            nc.vector.scalar_tensor_tensor(
                attn[:, w, :], rbc[:], 1.0, av_ps[:], AL.mult, AL.mult
            )

        if taps:
            nc.sync.dma_start(out=taps["tap_qT"], in_=qT[:].bitcast(F32))
            nc.sync.dma_start(out=taps["tap_kT"], in_=kT[:].bitcast(F32))
            if uniform:
                nc.sync.dma_start(out=taps["tap_P0"], in_=P0[:].bitcast(F32))
            nc.sync.dma_start(out=taps["tap_attn"], in_=attn[:].bitcast(F32))

        # ---------------- FF projection + ff_b ----------------
        for it in range(4):
            ps = ps_mm.tile([128, D], F32, tag="mmps")
            for w in range(2):
                nc.tensor.matmul(
                    ps[:],
                    attn[:, w, it * 128 : (it + 1) * 128],
                    ffwT[:, w, :],
                    start=(w == 0), stop=False,
                )
            nc.tensor.matmul(
                ps[:], ones_row[0:1, :], ffb_row[0:1, :], start=False, stop=True
            )
            ff_sb = stream.tile([128, D], F32, tag="t512")
            nc.vector.tensor_copy(ff_sb[:], ps[:])
            nc.sync.dma_start(
                out=t["d_ffpart"][it * 128 : (it + 1) * 128, :], in_=ff_sb[:]
            )
            if taps:
                nc.sync.dma_start(
                    out=taps["tap_ffpart"][it * 128 : (it + 1) * 128, :], in_=ff_sb[:]
                )

        # ---------------- pair ReduceScatter ----------------
        if no_collective:
            # timing-sim variant: replace the collective with a local copy
            for i in range(2):
                cp = stream.tile([128, D], F32, tag="t512")
                nc.sync.dma_start(out=cp[:], in_=t["d_ffpart"][i * 128 : (i + 1) * 128, :])
                nc.sync.dma_start(out=t["d_ffrs"][i * 128 : (i + 1) * 128, :], in_=cp[:])
        else:
            nc.gpsimd.collective_compute(
                "ReduceScatter",
                mybir.AluOpType.add,
                replica_groups=RS_GROUPS,
                ins=[t["d_ffpart"]],
                outs=[t["d_ffrs"]],
            )

        # ---------------- residual + LayerNorm on own 256 rows ----------------
        for ot in range(2):
            ff_ld = stream.tile([128, D], F32, tag="t512")
            nc.sync.dma_start(out=ff_ld[:], in_=t["d_ffrs"][ot * 128 : (ot + 1) * 128, :])
            x_ld = stream.tile([128, D], F32, tag="t512")
            nc.sync.dma_start(out=x_ld[:], in_=t["d_xrows"][ot * 128 : (ot + 1) * 128, :])
            y = stream.tile([128, D], F32, tag="t512")
            ysum = tiny.tile([128, 1], F32, tag="t1")
            nc.vector.scalar_tensor_tensor(
                y[:], x_ld[:], 1.0, ff_ld[:], AL.mult, AL.add, accum_out=ysum[:]
            )
            negmu = tiny.tile([128, 1], F32, tag="t1")
            nc.vector.tensor_scalar(negmu[:], ysum[:], -1.0 / D, None, AL.mult)
            sq = stream.tile([128, D], F32, tag="t512")
            ssq = tiny.tile([128, 1], F32, tag="t1")
            nc.scalar.activation(
                sq[:], y[:], AF.Square, bias=negmu[:], scale=1.0, accum_out=ssq[:]
            )
            veps = tiny.tile([128, 1], F32, tag="t1")
            nc.vector.tensor_scalar(veps[:], ssq[:], 1.0 / D, LN_EPS, AL.mult, AL.add)
            std = tiny.tile([128, 1], F32, tag="t1")
            nc.scalar.activation(std[:], veps[:], AF.Sqrt)
            rstd = tiny.tile([128, 1], F32, tag="t1")
            nc.vector.reciprocal(rstd[:], std[:])
            z = stream.tile([128, D], F32, tag="t512")
            nc.vector.tensor_scalar(z[:], y[:], negmu[:], rstd[:], AL.add, AL.mult)
            if trivial_ln:
                o = z
            else:
                zw = stream.tile([128, D], F32, tag="t512")
                nc.vector.scalar_tensor_tensor(zw[:], lnw_bc[:], 1.0, z[:], AL.mult, AL.mult)
                o = stream.tile([128, D], F32, tag="t512")
                nc.vector.scalar_tensor_tensor(o[:], lnb_bc[:], 1.0, zw[:], AL.mult, AL.add)
            nc.sync.dma_start(out=t["d_out"][ot * 128 : (ot + 1) * 128, :], in_=o[:])


_PROGRAM_CACHE = {}


def _get_program(uniform, gp, ga, trivial_ln):
    key = (uniform, float(gp), float(ga), trivial_ln)
    if key not in _PROGRAM_CACHE:
        _PROGRAM_CACHE[key] = build_program(uniform, gp, ga, trivial_ln)
    return _PROGRAM_CACHE[key]


def _shard_inputs(x, pdist, angle, adj, mask, gamma_p, gamma_adj, w_bias,
                  att_w, ff_w, ff_b, ln_w, ln_b):
    """Host-side sharding: slice + pre-transpose per-core inputs."""
    f32 = np.float32
    c_ = np.ascontiguousarray
    in_maps = []
    for c in range(N_CORES):
        b, ih, hg = c // 4, (c % 4) // 2, c % 2
        irows = slice(ih * NI, (ih + 1) * NI)
        orows = slice(ih * NI + hg * 256, ih * NI + hg * 256 + 256)
        hsl = slice(hg * NH, (hg + 1) * NH)          # heads
        fsl = slice(hg * 256, hg * 256 + 256)        # feature rows within q/k/v blocks
        awT = att_w.T  # [D, 3*H*DH]
        wqkT = np.concatenate([awT[:, 0:512][:, fsl], awT[:, 512:1024][:, fsl]], axis=1)
        hcoef = np.stack(
            [w_bias[hsl, 0], w_bias[hsl, 1], -gamma_p[hsl], gamma_adj[hsl]], axis=1
        )
        maskb = np.where(mask[b, 0, 0, :], np.float32(NEG_INF), np.float32(0.0))
        in_maps.append({
            "xT": c_(x[b].T, dtype=f32),
            "xiT": c_(x[b, irows].T, dtype=f32),
            "xrows": c_(x[b, orows], dtype=f32),
            "wqkT": c_(wqkT, dtype=f32),
            "wvT": c_(awT[:, 1024:1536][:, fsl], dtype=f32),
            "ffwT": c_(ff_w.T[hg * 256 : hg * 256 + 256, :], dtype=f32),
            "pdT": c_(pdist[b, irows, :].T, dtype=f32),
            "adT": c_(adj[b, irows, :].T, dtype=f32),
            "a0T": c_(angle[b, irows, :, 0].T, dtype=f32),
            "a1T": c_(angle[b, irows, :, 1].T, dtype=f32),
            "hcoef": c_(hcoef, dtype=f32),
            "maskb": c_(maskb, dtype=f32),
            "lnw": c_(ln_w, dtype=f32),
            "lnb": c_(ln_b, dtype=f32),
            "ffb": c_(ff_b, dtype=f32),
        })
    return in_maps


def kernel(x, pdist, angle, adj, mask, gamma_p, gamma_adj, w_bias,
           att_w, ff_w, ff_b, ln_w, ln_b, **_unused):
    x = np.asarray(x, dtype=np.float32)
    pdist = np.asarray(pdist, dtype=np.float32)
    angle = np.asarray(angle, dtype=np.float32)
    adj = np.asarray(adj, dtype=np.float32)
    mask = np.asarray(mask)
    gamma_p = np.asarray(gamma_p, dtype=np.float32)
    gamma_adj = np.asarray(gamma_adj, dtype=np.float32)
    w_bias = np.asarray(w_bias, dtype=np.float32)
    att_w = np.asarray(att_w, dtype=np.float32)
    ff_w = np.asarray(ff_w, dtype=np.float32)
    ff_b = np.asarray(ff_b, dtype=np.float32)
    ln_w = np.asarray(ln_w, dtype=np.float32)
    ln_b = np.asarray(ln_b, dtype=np.float32)

    uniform = bool(
        np.all(gamma_p == gamma_p.flat[0]) and np.all(gamma_adj == gamma_adj.flat[0])
    )
    gp = float(gamma_p.flat[0]) if uniform else 0.0
    ga = float(gamma_adj.flat[0]) if uniform else 0.0

    trivial_ln = bool(np.all(ln_w == 1.0) and np.all(ln_b == 0.0))
    nc = _get_program(uniform, gp, ga, trivial_ln)
    in_maps = _shard_inputs(x, pdist, angle, adj, mask, gamma_p, gamma_adj,
                            w_bias, att_w, ff_w, ff_b, ln_w, ln_b)
    res = run_bass_kernel_spmd(nc, in_maps, list(range(N_CORES)))

    out = np.empty((B, N, D), dtype=np.float32)
    for c in range(N_CORES):
        b, ih, hg = c // 4, (c % 4) // 2, c % 2
        r0 = ih * NI + hg * 256
        out[b, r0 : r0 + 256, :] = res.results[c]["out"]
    return out

